# revision 7
# baseline (speedup 1.0000x reference)
"""Multi-head attention forward, tensor-parallel over 8 TRN2 NeuronCores.

Problem: x[4,2048,1024], Wqkv[1024,3072], bqkv[3072], Wo[1024,1024], bo[1024]
  qkv = x @ Wqkv + bqkv ; 16 heads, d_head 64 ; softmax(QK^T/8) V ; out proj.

Sharding: DP=2 over batch (2 batches/core) x TP=4 over heads (4 heads/core).
Each core computes a partial y^T (its heads' contribution, transposed); the
host sums partials within each batch group, adds biases, and transposes.

Device dataflow (all-transposed to keep per-partition bias / avoid on-chip
transposes):
  qT,kT = (W_{q,k}^T x^T + b)      [256, 4096]  (feature-on-partition)
  v     = x W_v                    [4096, 256]  (natural; bias folded on host:
                                    P@(V+1 b_v^T) => y += (b_v @ Wo) on host)
  S^T   = K Q^T  per (batch,head)  strips [128k, 1024q] in PSUM
  P^T   = exp(S^T / 8)             ACT, no max-subtraction (|S/8| < ~2.5)
  O^T|s = [V|1]^T P^T              PV matmul with ones column -> row 64 = rowsum
  O_n^T = O^T * (1/s) broadcast    (e65 selector matmul broadcasts row 64)
  y^T  += Wo_part^T O_n^T          [1024, 4096] partial, summed on host

Matmuls run as float32r (full PE rate at free dim >= 256, fp32 storage).
"""

import sys

if "/opt/trn_rl_repo" not in sys.path:
    sys.path.insert(0, "/opt/trn_rl_repo")

import numpy as np

B, S, D = 4, 2048, 1024
H, DH = 16, 64
NCORES = 8
DP, TP = 2, 4
BL = B // DP            # 2 local batches
TOK = BL * S            # 4096 local tokens
HL = H // TP            # 4 local heads
HD = HL * DH            # 256 local head dims
P = 128
NF = D // P             # 8 feature chunks
NJ = TOK // 512         # 8 token chunks of 512
NKS = S // P            # 16 k-strips per (batch, head)
QB = 1024               # q block per exp call
NQH = S // QB           # 2 q blocks per pair

_cache = {}


def _build():
    import concourse.bass as bass
    import concourse.tile as tile
    from concourse import bacc, mybir
    from contextlib import ExitStack

    FP = mybir.dt.float32
    FR = mybir.dt.float32r
    AF = mybir.ActivationFunctionType

    nc = bacc.Bacc("TRN2", target_bir_lowering=False, debug=False,
                   num_devices=NCORES)

    xT = nc.dram_tensor("xT", [D, TOK], FR, kind="ExternalInput").ap()
    w = nc.dram_tensor("w", [D, 3 * HD], FR, kind="ExternalInput").ap()
    bqk = nc.dram_tensor("bqk", [2 * HD, 1], FP, kind="ExternalInput").ap()
    wo = nc.dram_tensor("wo", [HD, D], FR, kind="ExternalInput").ap()
    yT = nc.dram_tensor("yT", [D, TOK], FP, kind="ExternalOutput").ap()

    with tile.TileContext(nc) as tc, ExitStack() as ctx:
        konst = ctx.enter_context(tc.tile_pool(name="konst", bufs=1))
        xt_p = ctx.enter_context(tc.tile_pool(name="xt", bufs=2))
        stage = ctx.enter_context(tc.tile_pool(name="stage", bufs=3))
        pair_p = ctx.enter_context(tc.tile_pool(name="pair", bufs=2))
        pt_p = ctx.enter_context(tc.tile_pool(name="pt", bufs=3))
        rb_p = ctx.enter_context(tc.tile_pool(name="rb", bufs=2))
        ot_p = ctx.enter_context(tc.tile_pool(name="ot", bufs=1))
        on_p = ctx.enter_context(tc.tile_pool(name="on", bufs=2))
        mm_ps = ctx.enter_context(
            tc.tile_pool(name="mmps", bufs=2, space="PSUM"))
        s_ps = ctx.enter_context(
            tc.tile_pool(name="sps", bufs=2, space="PSUM"))
        pv_ps = ctx.enter_context(
            tc.tile_pool(name="pvps", bufs=2, space="PSUM"))
        dram = ctx.enter_context(
            tc.tile_pool(name="dram", bufs=1, space="DRAM"))

        # ---- constants resident in SBUF ----
        w_t = konst.tile([P, NF, 3 * HD], FR, tag="w")
        for f in range(NF):
            nc.sync.dma_start(w_t[:, f, :], w[f * P:(f + 1) * P, :])
        wo_t = konst.tile([P, 2, D], FR, tag="wo")
        for kc in range(2):
            nc.sync.dma_start(wo_t[:, kc, :], wo[kc * P:(kc + 1) * P, :])
        bias_t = konst.tile([P, 4], FP, tag="bias")
        for o in range(4):
            nc.sync.dma_start(bias_t[:, o:o + 1], bqk[o * P:(o + 1) * P, :])
        # e65: selects row 64 (the rowsum) in the broadcast matmul
        e65 = konst.tile([DH + 1, P], FP, tag="e65")
        nc.gpsimd.memset(e65[:], 0.0)
        nc.gpsimd.memset(e65[DH:DH + 1, :], 1.0)
        # reciprocal staging: row 64 written per (pair, qblock); rows 0..63
        # are a constant 1.0 so the e65 contraction stays finite
        rcp_t = konst.tile([DH + 1, 512], FP, tag="rcp")
        nc.gpsimd.memset(rcp_t[:], 1.0)
        # fp32 ones row used to fill the f32r vones column via DVE copy
        # (walrus rejects Memset on float32r APs)
        ones16 = konst.tile([P, NKS], FP, tag="ones16")
        nc.gpsimd.memset(ones16[:], 1.0)

        # ---- DRAM spill of qT/kT/v, split per local batch ----
        qTd = [dram.tile([HD, S], FR, tag=f"qTd{b}", name=f"qTd{b}")
               for b in range(BL)]
        kTd = [dram.tile([HD, S], FR, tag=f"kTd{b}", name=f"kTd{b}")
               for b in range(BL)]
        vNd = [dram.tile([S, HD], FR, tag=f"vNd{b}", name=f"vNd{b}")
               for b in range(BL)]

        # O^T (normalized), stacked 2 heads per tile: [128, 2048] x2 per batch
        ot_t = [[ot_p.tile([P, S], FR, tag=f"ot{b}_{kc}", name=f"ot{b}_{kc}")
                 for kc in range(2)] for b in range(BL)]

        def qkv_chunk(j):
            """Project token chunk j (512 tokens) -> qT/kT slices and v."""
            b = j // (NJ // BL)
            jj = j % (NJ // BL)          # chunk index within batch
            xt = xt_p.tile([P, NF, 512], FR, tag="xt")
            for f in range(NF):
                nc.sync.dma_start(
                    xt[:, f, :], xT[f * P:(f + 1) * P, j * 512:(j + 1) * 512])
            for o in range(4):           # q0 q1 k0 k1 (128 rows each)
                ps = mm_ps.tile([P, 512], FP, tag="mm")
                for f in range(NF):
                    nc.tensor.matmul(
                        ps[:], w_t[:, f, o * P:(o + 1) * P], xt[:, f, :],
                        start=(f == 0), stop=(f == NF - 1))
                qk_sb = stage.tile([P, 512], FR, tag="stage")
                nc.vector.tensor_scalar_add(qk_sb[:], ps[:], bias_t[:, o:o + 1])
                dst = qTd[b] if o < 2 else kTd[b]
                r = (o % 2) * P
                nc.sync.dma_start(
                    dst[r:r + P, jj * 512:(jj + 1) * 512], qk_sb[:])
            for m in range(4):           # v natural: [128 tok, 256]
                ps = mm_ps.tile([P, 512], FP, tag="mm")
                for f in range(NF):
                    nc.tensor.matmul(
                        ps[:, :HD], xt[:, f, m * P:(m + 1) * P],
                        w_t[:, f, 2 * HD:3 * HD],
                        start=(f == 0), stop=(f == NF - 1))
                v_sb = stage.tile([P, HD], FR, tag="stage")
                nc.vector.tensor_copy(v_sb[:], ps[:, :HD])
                r = jj * 512 + m * P
                nc.sync.dma_start(vNd[b][r:r + P, :], v_sb[:])

        def pair(b, h):
            """Attention for (local batch b, local head h)."""
            kt = pair_p.tile([DH, S], FR, tag="kt")
            nc.sync.dma_start(kt[:], kTd[b][h * DH:(h + 1) * DH, :])
            qt = pair_p.tile([DH, S], FR, tag="qt")
            nc.sync.dma_start(qt[:], qTd[b][h * DH:(h + 1) * DH, :])
            vo = pair_p.tile([P, NKS, DH + 1], FR, tag="vo")
            nc.sync.dma_start(
                vo[:, :, :DH],
                vNd[b][:, h * DH:(h + 1) * DH].rearrange(
                    "(ks p) c -> p ks c", p=P))
            nc.vector.tensor_copy(vo[:, :, DH], ones16[:])

            for qh in range(NQH):
                q0 = qh * QB
                pvs = [pv_ps.tile([DH + 1, 512], FP, tag="pv", name="pv")
                       for _ in range(QB // 512)]
                for ks in range(NKS):
                    sp = s_ps.tile([P, QB], FP, tag="s")
                    for qc in range(QB // 512):
                        nc.tensor.matmul(
                            sp[:, qc * 512:(qc + 1) * 512],
                            kt[:, ks * P:(ks + 1) * P],
                            qt[:, q0 + qc * 512:q0 + (qc + 1) * 512],
                            start=True, stop=True)
                    pt = pt_p.tile([P, QB], FR, tag="pt")
                    nc.scalar.activation(pt[:], sp[:], AF.Exp, scale=0.125)
                    for qc in range(QB // 512):
                        nc.tensor.matmul(
                            pvs[qc][:],
                            vo[:, ks, :], pt[:, qc * 512:(qc + 1) * 512],
                            start=(ks == 0), stop=(ks == NKS - 1))
                dst = ot_t[b][h // 2]
                for qc in range(QB // 512):
                    pv = pvs[qc]
                    c0 = q0 + qc * 512
                    # rowsum -> reciprocal (partition 64 aligned in and out)
                    nc.vector.reciprocal(rcp_t[DH:DH + 1, :], pv[DH:DH + 1, :])
                    rb = rb_p.tile([P, 512], FP, tag="rb")
                    bc = mm_ps.tile([P, 512], FP, tag="mm")
                    nc.tensor.matmul(bc[:], e65[:], rcp_t[:],
                                     start=True, stop=True)
                    nc.vector.tensor_copy(rb[:], bc[:])
                    if h % 2 == 0:
                        nc.vector.tensor_mul(
                            dst[0:DH, c0:c0 + 512], pv[0:DH, :], rb[0:DH, :])
                    else:
                        on = on_p.tile([DH, 512], FR, tag="on")
                        nc.vector.tensor_mul(on[:], pv[0:DH, :], rb[0:DH, :])
                        # partition shift (rows 64..127) via DMA
                        nc.sync.dma_start(dst[DH:2 * DH, c0:c0 + 512], on[:])

        def proj(b):
            """y^T partial for local batch b: [1024, 2048] block."""
            for fo in range(NF):
                for t4 in range(S // 512):
                    yp = mm_ps.tile([P, 512], FP, tag="mm")
                    for kc in range(2):
                        nc.tensor.matmul(
                            yp[:], wo_t[:, kc, fo * P:(fo + 1) * P],
                            ot_t[b][kc][:, t4 * 512:(t4 + 1) * 512],
                            start=(kc == 0), stop=(kc == 1))
                    y_sb = stage.tile([P, 512], FP, tag="stage")
                    nc.vector.tensor_copy(y_sb[:], yp[:])
                    nc.sync.dma_start(
                        yT[fo * P:(fo + 1) * P,
                           b * S + t4 * 512:b * S + (t4 + 1) * 512], y_sb[:])

        # ---- emission order: qkv(b0), then pairs(b0) interleaved with
        # qkv(b1), then proj(b0), pairs(b1), proj(b1) ----
        for j in range(0, 4):
            qkv_chunk(j)
        pair(0, 0)
        qkv_chunk(4)
        qkv_chunk(5)
        pair(0, 1)
        qkv_chunk(6)
        qkv_chunk(7)
        pair(0, 2)
        pair(0, 3)
        pair(1, 0)
        proj(0)
        for h in range(1, HL):
            pair(1, h)
        proj(1)

    nc.compile()
    return nc


def build():
    if "nc" not in _cache:
        _cache["nc"] = _build()
    return _cache["nc"]


def make_in_maps(x, Wqkv, bqkv, Wo):
    x = np.ascontiguousarray(np.asarray(x, np.float32))
    Wqkv = np.asarray(Wqkv, np.float32)
    bqkv = np.asarray(bqkv, np.float32)
    Wo = np.asarray(Wo, np.float32)
    in_maps = []
    for c in range(NCORES):
        g, t = divmod(c, TP)
        xTc = np.ascontiguousarray(
            x[g * BL:(g + 1) * BL].reshape(TOK, D).T)
        wc = np.ascontiguousarray(np.concatenate(
            [Wqkv[:, i * D + t * HD:i * D + (t + 1) * HD] for i in range(3)],
            axis=1))
        bqkc = np.ascontiguousarray(np.concatenate(
            [bqkv[t * HD:(t + 1) * HD],
             bqkv[D + t * HD:D + (t + 1) * HD]]).reshape(2 * HD, 1))
        woc = np.ascontiguousarray(Wo[t * HD:(t + 1) * HD, :])
        in_maps.append({"xT": xTc, "w": wc, "bqk": bqkc, "wo": woc})
    return in_maps


LAST_EXEC_NS = None


def kernel(x, Wqkv, bqkv, Wo, bo):
    global LAST_EXEC_NS
    from concourse import bass_utils

    nc = build()
    in_maps = make_in_maps(x, Wqkv, bqkv, Wo)
    res = bass_utils.run_bass_kernel_spmd(
        nc, in_maps, core_ids=list(range(NCORES)))
    LAST_EXEC_NS = res.exec_time_ns
    outs = [r["yT"] for r in res.results]

    Wqkv = np.asarray(Wqkv, np.float32)
    Wo = np.asarray(Wo, np.float32)
    bo = np.asarray(bo, np.float32)
    bqkv = np.asarray(bqkv, np.float32)
    hb = bo + np.asarray(bqkv[2 * D:3 * D], np.float32) @ Wo

    halves = []
    for g in range(DP):
        acc = outs[g * TP].astype(np.float32)
        for t in range(1, TP):
            acc = acc + outs[g * TP + t]
        halves.append(acc.T)            # [TOK, D]
    y = np.concatenate(halves, axis=0) + hb[None, :]
    return np.ascontiguousarray(y.reshape(B, S, D).astype(np.float32))


# revision 10
# speedup vs baseline: 1.1271x; 1.1271x over previous
"""Multi-head attention forward, tensor-parallel over 8 TRN2 NeuronCores.

Problem: x[4,2048,1024], Wqkv[1024,3072], bqkv[3072], Wo[1024,1024], bo[1024]
  qkv = x @ Wqkv + bqkv ; 16 heads, d_head 64 ; softmax(QK^T/8) V ; out proj.

Sharding: DP=2 over batch (2 batches/core) x TP=4 over heads (4 heads/core).
Each core computes a partial y^T (its heads' contribution, transposed); the
host sums partials within each batch group, adds biases, and transposes.

Device dataflow (all-transposed to keep per-partition bias / avoid on-chip
transposes):
  qT,kT = (W_{q,k}^T x^T + b)      [256, 4096]  (feature-on-partition)
  v     = x W_v                    [4096, 256]  (natural; bias folded on host:
                                    P@(V+1 b_v^T) => y += (b_v @ Wo) on host)
  S^T   = K Q^T  per (batch,head)  strips [128k, 1024q] in PSUM
  P^T   = exp(S^T / 8)             ACT, no max-subtraction (|S/8| < ~2.5)
  O^T|s = [V|1]^T P^T              PV matmul with ones column -> row 64 = rowsum
  O_n^T = O^T * (1/s) broadcast    (e65 selector matmul broadcasts row 64)
  y^T  += Wo_part^T O_n^T          [1024, 4096] partial, summed on host

Matmuls run as float32r (full PE rate at free dim >= 256, fp32 storage).
"""

import sys

if "/opt/trn_rl_repo" not in sys.path:
    sys.path.insert(0, "/opt/trn_rl_repo")

import numpy as np

B, S, D = 4, 2048, 1024
H, DH = 16, 64
NCORES = 8
DP, TP = 2, 4
BL = B // DP            # 2 local batches
TOK = BL * S            # 4096 local tokens
HL = H // TP            # 4 local heads
HD = HL * DH            # 256 local head dims
P = 128
NF = D // P             # 8 feature chunks
NJ = TOK // 512         # 8 token chunks of 512
NKS = S // P            # 16 k-strips per (batch, head)
QB = 1024               # q block per exp call
NQH = S // QB           # 2 q blocks per pair

USE_BF16 = True          # matmul operand dtype: bf16 vs float32r

_cache = {}


def _build():
    import concourse.bass as bass
    import concourse.tile as tile
    from concourse import bacc, mybir
    from contextlib import ExitStack

    FP = mybir.dt.float32
    FR = mybir.dt.bfloat16 if USE_BF16 else mybir.dt.float32r
    NMOV = 512   # moving free dim per matmul (PSUM output bank limit)
    AF = mybir.ActivationFunctionType

    nc = bacc.Bacc("TRN2", target_bir_lowering=False, debug=False,
                   num_devices=NCORES)

    xT = nc.dram_tensor("xT", [D, TOK], FR, kind="ExternalInput").ap()
    w = nc.dram_tensor("w", [D, 3 * HD], FR, kind="ExternalInput").ap()
    bqk = nc.dram_tensor("bqk", [2 * HD, 1], FP, kind="ExternalInput").ap()
    wo = nc.dram_tensor("wo", [HD, D], FR, kind="ExternalInput").ap()
    yT = nc.dram_tensor("yT", [D, TOK], FP, kind="ExternalOutput").ap()

    with tile.TileContext(nc) as tc, ExitStack() as ctx:
        konst = ctx.enter_context(tc.tile_pool(name="konst", bufs=1))
        xt_p = ctx.enter_context(tc.tile_pool(name="xt", bufs=2))
        stage = ctx.enter_context(tc.tile_pool(name="stage", bufs=3))
        pair_p = ctx.enter_context(tc.tile_pool(name="pair", bufs=2))
        pt_p = ctx.enter_context(tc.tile_pool(name="pt", bufs=3))
        rb_p = ctx.enter_context(tc.tile_pool(name="rb", bufs=2))
        ot_p = ctx.enter_context(tc.tile_pool(name="ot", bufs=1))
        on_p = ctx.enter_context(tc.tile_pool(name="on", bufs=2))
        mm_ps = ctx.enter_context(
            tc.tile_pool(name="mmps", bufs=2, space="PSUM"))
        s_ps = ctx.enter_context(
            tc.tile_pool(name="sps", bufs=2, space="PSUM"))
        pv_ps = ctx.enter_context(
            tc.tile_pool(name="pvps", bufs=2, space="PSUM"))
        dram = ctx.enter_context(
            tc.tile_pool(name="dram", bufs=1, space="DRAM"))

        # ---- constants resident in SBUF ----
        w_t = konst.tile([P, NF, 3 * HD], FR, tag="w")
        for f in range(NF):
            nc.sync.dma_start(w_t[:, f, :], w[f * P:(f + 1) * P, :])
        wo_t = konst.tile([P, 2, D], FR, tag="wo")
        for kc in range(2):
            nc.sync.dma_start(wo_t[:, kc, :], wo[kc * P:(kc + 1) * P, :])
        bias_t = konst.tile([P, 4], FP, tag="bias")
        for o in range(4):
            nc.sync.dma_start(bias_t[:, o:o + 1], bqk[o * P:(o + 1) * P, :])
        # e65: selects row 64 (the rowsum) in the broadcast matmul
        e65 = konst.tile([DH + 1, P], FP, tag="e65")
        nc.gpsimd.memset(e65[:], 0.0)
        nc.gpsimd.memset(e65[DH:DH + 1, :], 1.0)
        # reciprocal staging: row 64 written per (pair, qblock); rows 0..63
        # are a constant 1.0 so the e65 contraction stays finite
        rcp_t = konst.tile([DH + 1, 512], FP, tag="rcp")
        nc.gpsimd.memset(rcp_t[:], 1.0)
        # fp32 ones row used to fill the f32r vones column via DVE copy
        # (walrus rejects Memset on float32r APs)
        ones16 = konst.tile([P, NKS], FP, tag="ones16")
        nc.gpsimd.memset(ones16[:], 1.0)

        # ---- DRAM spill of qT/kT/v, split per local batch ----
        qTd = [dram.tile([HD, S], FR, tag=f"qTd{b}", name=f"qTd{b}")
               for b in range(BL)]
        kTd = [dram.tile([HD, S], FR, tag=f"kTd{b}", name=f"kTd{b}")
               for b in range(BL)]
        vNd = [dram.tile([S, HD], FR, tag=f"vNd{b}", name=f"vNd{b}")
               for b in range(BL)]

        # O^T (normalized), stacked 2 heads per tile: [128, 2048] x2 per batch
        ot_t = [[ot_p.tile([P, S], FR, tag=f"ot{b}_{kc}", name=f"ot{b}_{kc}")
                 for kc in range(2)] for b in range(BL)]

        def qkv_chunk(j):
            """Project token chunk j (512 tokens) -> qT/kT slices and v."""
            b = j // (NJ // BL)
            jj = j % (NJ // BL)          # chunk index within batch
            xt = xt_p.tile([P, NF, 512], FR, tag="xt")
            for f in range(NF):
                nc.sync.dma_start(
                    xt[:, f, :], xT[f * P:(f + 1) * P, j * 512:(j + 1) * 512])
            for o in range(4):           # q0 q1 k0 k1 (128 rows each)
                ps = mm_ps.tile([P, 512], FP, tag="mm")
                for f in range(NF):
                    nc.tensor.matmul(
                        ps[:], w_t[:, f, o * P:(o + 1) * P], xt[:, f, :],
                        start=(f == 0), stop=(f == NF - 1))
                qk_sb = stage.tile([P, 512], FR, tag="stage")
                nc.vector.tensor_scalar_add(qk_sb[:], ps[:], bias_t[:, o:o + 1])
                dst = qTd[b] if o < 2 else kTd[b]
                r = (o % 2) * P
                nc.sync.dma_start(
                    dst[r:r + P, jj * 512:(jj + 1) * 512], qk_sb[:])
            for m in range(4):           # v natural: [128 tok, 256]
                ps = mm_ps.tile([P, 512], FP, tag="mm")
                for f in range(NF):
                    nc.tensor.matmul(
                        ps[:, :HD], xt[:, f, m * P:(m + 1) * P],
                        w_t[:, f, 2 * HD:3 * HD],
                        start=(f == 0), stop=(f == NF - 1))
                v_sb = stage.tile([P, HD], FR, tag="stage")
                nc.vector.tensor_copy(v_sb[:], ps[:, :HD])
                r = jj * 512 + m * P
                nc.sync.dma_start(vNd[b][r:r + P, :], v_sb[:])

        def pair(b, h):
            """Attention for (local batch b, local head h)."""
            kt = pair_p.tile([DH, S], FR, tag="kt")
            nc.sync.dma_start(kt[:], kTd[b][h * DH:(h + 1) * DH, :])
            qt = pair_p.tile([DH, S], FR, tag="qt")
            nc.sync.dma_start(qt[:], qTd[b][h * DH:(h + 1) * DH, :])
            vo = pair_p.tile([P, NKS, DH + 1], FR, tag="vo")
            nc.sync.dma_start(
                vo[:, :, :DH],
                vNd[b][:, h * DH:(h + 1) * DH].rearrange(
                    "(ks p) c -> p ks c", p=P))
            nc.vector.tensor_copy(vo[:, :, DH], ones16[:])

            for qh in range(NQH):
                q0 = qh * QB
                pvs = [pv_ps.tile([DH + 1, 512], FP, tag="pv", name="pv")
                       for _ in range(QB // 512)]
                for ks in range(NKS):
                    sp = s_ps.tile([P, QB], FP, tag="s")
                    for qc in range(QB // NMOV):
                        nc.tensor.matmul(
                            sp[:, qc * NMOV:(qc + 1) * NMOV],
                            kt[:, ks * P:(ks + 1) * P],
                            qt[:, q0 + qc * NMOV:q0 + (qc + 1) * NMOV],
                            start=True, stop=True)
                    pt = pt_p.tile([P, QB], FR, tag="pt")
                    nc.scalar.activation(pt[:], sp[:], AF.Exp, scale=0.125)
                    for qc in range(QB // 512):
                        nc.tensor.matmul(
                            pvs[qc][:],
                            vo[:, ks, :], pt[:, qc * 512:(qc + 1) * 512],
                            start=(ks == 0), stop=(ks == NKS - 1))
                dst = ot_t[b][h // 2]
                for qc in range(QB // 512):
                    pv = pvs[qc]
                    c0 = q0 + qc * 512
                    # rowsum -> reciprocal (partition 64 aligned in and out)
                    nc.vector.reciprocal(rcp_t[DH:DH + 1, :], pv[DH:DH + 1, :])
                    rb = rb_p.tile([P, 512], FP, tag="rb")
                    bc = mm_ps.tile([P, 512], FP, tag="mm")
                    nc.tensor.matmul(bc[:], e65[:], rcp_t[:],
                                     start=True, stop=True)
                    nc.vector.tensor_copy(rb[:], bc[:])
                    if h % 2 == 0:
                        nc.vector.tensor_mul(
                            dst[0:DH, c0:c0 + 512], pv[0:DH, :], rb[0:DH, :])
                    else:
                        on = on_p.tile([DH, 512], FR, tag="on")
                        nc.vector.tensor_mul(on[:], pv[0:DH, :], rb[0:DH, :])
                        # partition shift (rows 64..127) via DMA
                        nc.sync.dma_start(dst[DH:2 * DH, c0:c0 + 512], on[:])

        def proj(b):
            """y^T partial for local batch b: [1024, 2048] block."""
            for fo in range(NF):
                for t4 in range(S // 512):
                    yp = mm_ps.tile([P, 512], FP, tag="mm")
                    for kc in range(2):
                        nc.tensor.matmul(
                            yp[:], wo_t[:, kc, fo * P:(fo + 1) * P],
                            ot_t[b][kc][:, t4 * 512:(t4 + 1) * 512],
                            start=(kc == 0), stop=(kc == 1))
                    y_sb = stage.tile([P, 512], FP, tag="stage")
                    nc.vector.tensor_copy(y_sb[:], yp[:])
                    nc.sync.dma_start(
                        yT[fo * P:(fo + 1) * P,
                           b * S + t4 * 512:b * S + (t4 + 1) * 512], y_sb[:])

        # ---- emission order: qkv(b0), then pairs(b0) interleaved with
        # qkv(b1), then proj(b0), pairs(b1), proj(b1) ----
        for j in range(0, 4):
            qkv_chunk(j)
        pair(0, 0)
        qkv_chunk(4)
        qkv_chunk(5)
        pair(0, 1)
        qkv_chunk(6)
        qkv_chunk(7)
        pair(0, 2)
        pair(0, 3)
        pair(1, 0)
        proj(0)
        for h in range(1, HL):
            pair(1, h)
        proj(1)

    nc.compile()
    return nc


def build():
    if "nc" not in _cache:
        _cache["nc"] = _build()
    return _cache["nc"]


def make_in_maps(x, Wqkv, bqkv, Wo):
    x = np.ascontiguousarray(np.asarray(x, np.float32))
    Wqkv = np.asarray(Wqkv, np.float32)
    bqkv = np.asarray(bqkv, np.float32)
    Wo = np.asarray(Wo, np.float32)
    if USE_BF16:
        import ml_dtypes
        mmdt = ml_dtypes.bfloat16
    else:
        mmdt = np.float32
    in_maps = []
    for c in range(NCORES):
        g, t = divmod(c, TP)
        xTc = np.ascontiguousarray(
            x[g * BL:(g + 1) * BL].reshape(TOK, D).T.astype(mmdt))
        wc = np.ascontiguousarray(np.concatenate(
            [Wqkv[:, i * D + t * HD:i * D + (t + 1) * HD] for i in range(3)],
            axis=1).astype(mmdt))
        bqkc = np.ascontiguousarray(np.concatenate(
            [bqkv[t * HD:(t + 1) * HD],
             bqkv[D + t * HD:D + (t + 1) * HD]]).reshape(2 * HD, 1))
        woc = np.ascontiguousarray(Wo[t * HD:(t + 1) * HD, :].astype(mmdt))
        in_maps.append({"xT": xTc, "w": wc, "bqk": bqkc, "wo": woc})
    return in_maps


LAST_EXEC_NS = None


def kernel(x, Wqkv, bqkv, Wo, bo):
    global LAST_EXEC_NS
    from concourse import bass_utils

    nc = build()
    in_maps = make_in_maps(x, Wqkv, bqkv, Wo)
    res = bass_utils.run_bass_kernel_spmd(
        nc, in_maps, core_ids=list(range(NCORES)))
    LAST_EXEC_NS = res.exec_time_ns
    outs = [r["yT"] for r in res.results]

    Wqkv = np.asarray(Wqkv, np.float32)
    Wo = np.asarray(Wo, np.float32)
    bo = np.asarray(bo, np.float32)
    bqkv = np.asarray(bqkv, np.float32)
    hb = bo + np.asarray(bqkv[2 * D:3 * D], np.float32) @ Wo

    halves = []
    for g in range(DP):
        acc = outs[g * TP].astype(np.float32)
        for t in range(1, TP):
            acc = acc + outs[g * TP + t]
        halves.append(acc.T)            # [TOK, D]
    y = np.concatenate(halves, axis=0) + hb[None, :]
    return np.ascontiguousarray(y.reshape(B, S, D).astype(np.float32))
